# revision 1
# baseline (speedup 1.0000x reference)
"""Trainium2 Bass kernel for the mixed low-rank-expert DCN-v2 block (nn_DCN_51539607711).

Reference math (L=3 layers, E=4 experts, D=512, R=64, B=16384):
  x_{l+1} = sum_e x0 * (tanh(tanh(x_l V_e) C_e) U_e^T + b_l) * gate_e + x_l
The gate is softmax over a size-1 axis == exactly 1.0, so G never affects the
output. With gate == 1 the recurrence telescopes:
  x_{l+1} = x0 * (1 + sum_{i<=l} (A_i(x_i) + E*b_i)),
  A_i(x) = sum_e U_e tanh(C_e^T tanh(V_e^T x))
so the residual stream is carried as a single PSUM accumulator s = sum_i A_i
(fp32, accumulated by the PE across all experts AND layers), and each layer's
activation update is ONE fused DVE op per chunk:
  x_{l+1} = (s + c_l[d]) * x0,   c_l = 1 + E*cumsum(b)_l   (per-partition scalar)

Distribution: pure data-parallel over B across 8 cores (2048 rows/core),
weights replicated. Activations live feature-major (xT: [D, B]) so every
matmul contracts on the partition dim with zero on-device transposes; the
host pre-transposes x and pre-packs weights (experts packed in pairs to fill
all 128 partitions/output rows):
  v-step :  vT[pair]  = Vpair^T  @ xlT    (lhsT = Vpair [D,128], K=D in 4 chunks)
  cv-step:  cvT[pair] = blockdiag(C_e0,C_e1)^T @ vT[pair]   (K=128)
  ucv    :  s[mchunk] += Upair^T-packed @ cvT[pair]          (K=128, accum)
All matmul operands bf16 (fp32 PSUM accumulation); residual + output fp32.
Per-core B is processed in blocks of 512 columns so s (4 PSUM banks) plus
matmul transients (4 banks) exactly fill PSUM.
"""

import numpy as np
import ml_dtypes

import concourse.bacc as bacc
import concourse.tile as tile
from concourse import mybir
from concourse.bass_utils import run_bass_kernel_spmd

L, E, D, R, B = 3, 4, 512, 64, 16384
NCORES = 8
BC = B // NCORES          # batch columns per core (2048)
NB = 512                  # block of batch columns (one PSUM bank at fp32)
NBLK = BC // NB           # blocks per core
P = 128                   # partitions
KC = D // P               # contraction chunks over D (4)
NPAIR = E // 2            # expert pairs (2)

F32 = mybir.dt.float32
BF16 = mybir.dt.bfloat16
bf16 = ml_dtypes.bfloat16

VW_COLS = L * NPAIR * KC * P      # 3072
CW_COLS = L * NPAIR * P           # 768
UW_COLS = L * NPAIR * D           # 3072
WB_COLS = VW_COLS + CW_COLS + UW_COLS

_CACHE = {}


def _build_nc(bc=BC):
    """Build the per-core Bass/Tile kernel. Identical NEFF on all cores."""
    nblk = bc // NB
    nc = bacc.Bacc("TRN2", target_bir_lowering=False, debug=False,
                   num_devices=NCORES)

    xf_d = nc.dram_tensor("xf", [D, bc], F32, kind="ExternalInput")
    xb_d = nc.dram_tensor("xb", [D, bc], BF16, kind="ExternalInput")
    # all bf16 weights in one blob: [vw | cw | uw] along the free dim
    # (one DMA — each dma_start costs ~0.6us of serial issue time)
    wb_d = nc.dram_tensor("wb", [P, WB_COLS], BF16, kind="ExternalInput")
    cb_d = nc.dram_tensor("cb", [P, L, KC], F32, kind="ExternalInput")
    out_d = nc.dram_tensor("out_t", [D, bc], F32, kind="ExternalOutput")

    # partition-major views of the feature-major x/out tensors
    xf_v = xf_d[:].rearrange("(k p) b -> p k b", p=P)
    xb_v = xb_d[:].rearrange("(k p) b -> p k b", p=P)
    out_v = out_d[:].rearrange("(m p) b -> p m b", p=P)

    Tanh = mybir.ActivationFunctionType.Tanh
    ADD = mybir.AluOpType.add
    MULT = mybir.AluOpType.mult

    with tile.TileContext(nc) as tc:
        with (
            tc.tile_pool(name="wpool", bufs=1) as wpool,
            tc.tile_pool(name="xpool", bufs=1) as xpool,
            tc.tile_pool(name="xl_pool", bufs=24) as xl_pool,
            tc.tile_pool(name="act_pool", bufs=14) as act_pool,
            tc.tile_pool(name="out_pool", bufs=6) as out_pool,
            tc.tile_pool(name="psum_s", bufs=4, space="PSUM") as psum_s,
            tc.tile_pool(name="psum_t", bufs=4, space="PSUM") as psum_t,
        ):
            # ---- persistent weights (host pre-packed into one blob);
            # region-split DMAs so the v weights arrive first ----
            wb_s = wpool.tile([P, WB_COLS], BF16)
            nc.sync.dma_start(wb_s[:, 0:VW_COLS], wb_d[:, 0:VW_COLS])
            xf_s = xpool.tile([P, KC, bc], F32)
            xb_s = xpool.tile([P, KC, bc], BF16)

            nc.sync.dma_start(wb_s[:, VW_COLS:VW_COLS + CW_COLS],
                              wb_d[:, VW_COLS:VW_COLS + CW_COLS])
            nc.sync.dma_start(wb_s[:, VW_COLS + CW_COLS:],
                              wb_d[:, VW_COLS + CW_COLS:])
            cb_s = wpool.tile([P, L, KC], F32)
            nc.sync.dma_start(cb_s[:], cb_d[:])

            vw_s = wb_s[:, 0:VW_COLS].rearrange(
                "p (l q k m) -> p l q k m", l=L, q=NPAIR, k=KC)
            cw_s = wb_s[:, VW_COLS:VW_COLS + CW_COLS].rearrange(
                "p (l q m) -> p l q m", l=L, q=NPAIR)
            uw_s = wb_s[:, VW_COLS + CW_COLS:].rearrange(
                "p (l q m) -> p l q m", l=L, q=NPAIR)

            for b in range(nblk):
                bs = slice(b * NB, (b + 1) * NB)
                # just-in-time per-chunk x loads for this block (small
                # contiguous-row DMAs issue fast and land early)
                for k in range(KC):
                    nc.sync.dma_start(xb_s[:, k, bs], xb_d[k * P:(k + 1) * P, bs])
                for k in range(KC):
                    nc.sync.dma_start(xf_s[:, k, bs], xf_d[k * P:(k + 1) * P, bs])

                s_tiles = [psum_s.tile([P, NB], F32, name=f"s_{b}_{m}", tag="s")
                           for m in range(KC)]
                xl_cur = [xb_s[:, k, bs] for k in range(KC)]

                for l in range(L):
                    # v = tanh(Vpair^T @ xl), one [128, NB] tile per expert pair
                    vts = []
                    for p in range(NPAIR):
                        vps = psum_t.tile([P, NB], F32, name=f"vps_{b}_{l}_{p}",
                                          tag="pst")
                        for k in range(KC):
                            nc.tensor.matmul(vps[:], vw_s[:, l, p, k, :],
                                             xl_cur[k],
                                             start=(k == 0), stop=(k == KC - 1))
                        vt = act_pool.tile([P, NB], BF16, name=f"vt_{b}_{l}_{p}",
                                           tag="act")
                        nc.scalar.activation(vt[:], vps[:], Tanh)
                        vts.append(vt)
                    # cv = tanh(blockdiag(C)^T @ v)
                    cvts = []
                    for p in range(NPAIR):
                        cps = psum_t.tile([P, NB], F32, name=f"cps_{b}_{l}_{p}",
                                          tag="pst")
                        nc.tensor.matmul(cps[:], cw_s[:, l, p, :], vts[p][:],
                                         start=True, stop=True)
                        cvt = act_pool.tile([P, NB], BF16, name=f"cvt_{b}_{l}_{p}",
                                            tag="act")
                        nc.scalar.activation(cvt[:], cps[:], Tanh)
                        cvts.append(cvt)
                    # s[m] += Upacked^T @ cv  (accumulates across pairs AND
                    # layers). Two m-chunks share each PSUM bank: start=True
                    # clears has_written for the WHOLE bank, so only the
                    # bank's first matmul (l0, p0, even m) starts; the odd
                    # chunk's first write lands on cleared bits and
                    # overwrites, everything else accumulates. stop closes
                    # the sim's group per bank so the DVE may read s; later
                    # layers bypass the sim group check (HW accumulates via
                    # per-element has_written bits regardless).
                    for m in range(KC):
                        for p in range(NPAIR):
                            nc.tensor.matmul(
                                s_tiles[m],
                                uw_s[:, l, p, m * P:(m + 1) * P],
                                cvts[p][:],
                                start=(l == 0 and p == 0),
                                stop=(l == 0 and p == NPAIR - 1),
                                skip_group_check=(l > 0),
                            )
                    # x_{l+1} = (s + c_l) * x0.  Chunks 0,1 via one fused DVE
                    # op; chunks 2,3 via ACT Identity(+c) -> cheap bf16 DVE
                    # mul, so xl production runs on two engines concurrently
                    # instead of serializing 4 STTs on the DVE.
                    if l < L - 1:
                        nxt = []
                        for m in range(KC):
                            xln = xl_pool.tile([P, NB], BF16,
                                               name=f"xl_{b}_{l}_{m}", tag="xl")
                            if m < 2:
                                nc.vector.scalar_tensor_tensor(
                                    xln[:], s_tiles[m], cb_s[:, l, m:m + 1],
                                    xf_s[:, m, bs], ADD, MULT)
                            else:
                                um = act_pool.tile([P, NB], BF16,
                                                   name=f"u_{b}_{l}_{m}",
                                                   tag="act")
                                nc.scalar.activation(
                                    um[:], s_tiles[m],
                                    mybir.ActivationFunctionType.Identity,
                                    bias=cb_s[:, l, m:m + 1])
                                nc.vector.tensor_mul(
                                    xln[:], um[:], xb_s[:, m, bs])
                            nxt.append(xln)
                        xl_cur = [t[:] for t in nxt]
                    else:
                        # final layer: same dual-engine production (chunks
                        # 2,3 via ACT Identity + DVE mul, fp32), and store
                        # in two halves so the first DMA starts early
                        ot = out_pool.tile([P, KC, NB], F32,
                                           name=f"ot_{b}", tag="ot")
                        for m in range(KC):
                            if m < 2:
                                nc.vector.scalar_tensor_tensor(
                                    ot[:, m, :], s_tiles[m],
                                    cb_s[:, l, m:m + 1],
                                    xf_s[:, m, bs], ADD, MULT)
                            else:
                                uo = out_pool.tile([P, NB], F32,
                                                   name=f"uo_{b}_{m}",
                                                   tag="uo")
                                nc.scalar.activation(
                                    uo[:], s_tiles[m],
                                    mybir.ActivationFunctionType.Identity,
                                    bias=cb_s[:, l, m:m + 1])
                                nc.vector.tensor_mul(
                                    ot[:, m, :], uo[:], xf_s[:, m, bs])
                            if m == 1:
                                nc.sync.dma_start(out_v[:, 0:2, bs],
                                                  ot[:, 0:2, :])
                        nc.sync.dma_start(out_v[:, 2:4, bs], ot[:, 2:4, :])

    nc.compile()
    return nc


def _prep_weights(U, V, C, bias):
    """Host-side packing into the exact SBUF layouts (see module docstring)."""
    VwH = np.empty([P, L, NPAIR, KC, P], dtype=bf16)
    UwH = np.empty([P, L, NPAIR, D], dtype=bf16)
    CwH = np.zeros([P, L, NPAIR, P], dtype=bf16)
    for l in range(L):
        for p in range(NPAIR):
            vpair = np.concatenate([V[l, 2 * p], V[l, 2 * p + 1]], axis=1)  # [D,128]
            VwH[:, l, p, :, :] = vpair.reshape(KC, P, P).transpose(1, 0, 2)
            upair = np.concatenate([U[l, 2 * p].T, U[l, 2 * p + 1].T], axis=0)  # [128,D]
            UwH[:, l, p, :] = upair
            CwH[:R, l, p, :R] = C[l, 2 * p]
            CwH[R:, l, p, R:] = C[l, 2 * p + 1]
    wb = np.concatenate([VwH.reshape(P, VW_COLS), CwH.reshape(P, CW_COLS),
                         UwH.reshape(P, UW_COLS)], axis=1)
    cb = 1.0 + E * np.cumsum(bias.astype(np.float32), axis=0)       # [L, D]
    cbH = np.ascontiguousarray(
        cb.reshape(L, KC, P).transpose(2, 0, 1)).astype(np.float32)  # [P, L, KC]
    return np.ascontiguousarray(wb), cbH


def _make_in_maps(x, U, V, C, G, bias):
    wbH, cbH = _prep_weights(np.asarray(U, np.float32),
                             np.asarray(V, np.float32),
                             np.asarray(C, np.float32),
                             np.asarray(bias, np.float32))
    xT = np.asarray(x, np.float32).T                    # [D, B]
    in_maps = []
    for c in range(NCORES):
        xf = np.ascontiguousarray(xT[:, c * BC:(c + 1) * BC])
        in_maps.append({
            "xf": xf,
            "xb": xf.astype(bf16),
            "wb": wbH, "cb": cbH,
        })
    return in_maps


def _run(inputs, trace=False, **kw):
    key = "nc"
    if key not in _CACHE:
        _CACHE[key] = _build_nc()
    nc = _CACHE[key]
    in_maps = _make_in_maps(**inputs)
    res = run_bass_kernel_spmd(nc, in_maps, core_ids=list(range(NCORES)),
                               trace=trace, **kw)
    out = np.empty((B, D), np.float32)
    for c in range(NCORES):
        out[c * BC:(c + 1) * BC, :] = res.results[c]["out_t"].T
    return out, res


def kernel(**inputs) -> np.ndarray:
    out, _ = _run(inputs, trace=False)
    return out

